# revision 34
# baseline (speedup 1.0000x reference)
"""Self-contained Trainium2 Bass kernel: 16-head causal attention with RoPE.

Sharding: DP2 x TP4 — core c handles batch c//4 and heads [4*(c%4), 4*(c%4)+4).
Each core computes the qkv projection for its batch/heads, causal flash
attention, and a partial output projection (w_o columns for its heads); the
4 partial [L, H] outputs per batch are summed on the host.

Key layout/engine strategy:
  - All matmul operands are bf16 (PE streams 1 row/cycle at ANY moving size;
    f32r drops to 4 cycles/row below 256). PSUM accumulation stays fp32, so
    the only precision loss is bf16 operand quantization (~0.4% rel), well
    inside the 2e-2 gate.
  - q, k computed head-dim-major qT/kT [128, L]; v token-major [L, 128] with
    a 129th column of ones.
  - RoPE entirely on DVE: the sin/cos tables repeat rows at +64, so the
    half-rotation is a partition-shifted write of the sin product (legal on
    HW because the multiply reads PSUM; only SBUF+SBUF operand pairs must
    share a partition base), then base-aligned sub/add. No matmul needed.
  - scores computed transposed scT [k_tok, q_tok] = kT_chunk.T @ qT per
    256-token q stripe, two k-chunks packed per PSUM bank so each Exp
    activation covers [128, 512] (amortizes ACT per-instruction overhead).
  - attnV + softmax denominator FUSED: out[q, 0:129] = e_chunk.T @ [v | 1]
    accumulated over k chunks. Column 128 is the denominator, so
    normalization is a per-partition reciprocal + tensor_scalar multiply on
    DVE — no ones-matmul pass, no broadcast matmul.
  - normalized o is transposed back to head-dim-major via an XBAR DMA
    transpose (SBUF->SBUF, no PE or PSUM involvement) so it slots directly
    into the w_o projection as the stationary operand.
  - output projection matmuls are interleaved into the NEXT stripe's
    attention loop: they give the PE work to do while ACT computes Exp.
"""

import numpy as np
from contextlib import ExitStack

import concourse.bass as bass
import concourse.tile as tile
from concourse import bacc, mybir
from concourse.bass_utils import run_bass_kernel_spmd
from concourse.masks import make_upper_triangular

F32 = mybir.dt.float32
BF16 = mybir.dt.bfloat16
AF = mybir.ActivationFunctionType

NCORES = 8
DP = 2          # batch groups
TP = 4          # head groups per batch
HD = 128
ROPE_THETA = 10000.0


def rope_tables_T(Lsz):
    """cos/sin tables transposed to [HD, L], matching the fp32 reference."""
    half = np.arange(0, HD, 2).astype(np.float32) / np.float32(HD)
    inv_freq = (np.float32(1.0) / np.power(np.float32(ROPE_THETA), half,
                                           dtype=np.float32)).astype(np.float32)
    t = np.arange(Lsz, dtype=np.float32)
    freqs = np.outer(t, inv_freq).astype(np.float32)          # [L, HD/2]
    emb = np.concatenate([freqs, freqs], axis=1)              # [L, HD]
    cosT = np.ascontiguousarray(np.cos(emb).astype(np.float32).T)  # [HD, L]
    sinT = np.ascontiguousarray(np.sin(emb).astype(np.float32).T)
    return cosT, sinT


def build_attention_nc(Lsz, Hsz, hpc, repeat=1, phases=(1, 1, 1)):
    """Build + compile the per-core Bass program (identical on all cores).

    Each core: 1 batch of Lsz tokens, hpc heads. repeat>1 re-emits the whole
    computation N times in one program — used only for timing (wall-time
    slope isolates device exec from dispatch overhead)."""
    f = Hsz // 128            # feature chunks of the model dim
    dloc = hpc * HD           # local head dims
    RC = 512                  # token chunk for projection + rope
    QT = 256                  # q stripe for attention (2 x 128 sub-tiles)
    KCL = Lsz // 128          # k chunks per sequence
    NST = Lsz // QT           # stripes
    scale = float(1.0 / np.sqrt(HD))

    nc = bacc.Bacc("TRN2", target_bir_lowering=False, debug=False)

    xT = nc.dram_tensor("xT", [Hsz, Lsz], BF16, kind="ExternalInput").ap()
    wqT = nc.dram_tensor("wqT", [Hsz, dloc], BF16, kind="ExternalInput").ap()
    wkT = nc.dram_tensor("wkT", [Hsz, dloc], BF16, kind="ExternalInput").ap()
    wvT = nc.dram_tensor("wvT", [Hsz, dloc], BF16, kind="ExternalInput").ap()
    woT = nc.dram_tensor("woT", [dloc, Hsz], BF16, kind="ExternalInput").ap()
    cosT = nc.dram_tensor("cosT", [HD, Lsz], F32, kind="ExternalInput").ap()
    sinT = nc.dram_tensor("sinT", [HD, Lsz], F32, kind="ExternalInput").ap()
    y = nc.dram_tensor("y", [Lsz, Hsz], F32, kind="ExternalOutput").ap()

    with tile.TileContext(nc) as tc, \
         nc.allow_low_precision(reason="bf16 matmul operands"), ExitStack() as ctx:
        wpool = ctx.enter_context(tc.tile_pool(name="wpool", bufs=1))
        cpool = ctx.enter_context(tc.tile_pool(name="cpool", bufs=1))
        xpool = ctx.enter_context(tc.tile_pool(name="xpool", bufs=2))
        spool = ctx.enter_context(tc.tile_pool(name="spool", bufs=1))
        work = ctx.enter_context(tc.tile_pool(name="work", bufs=2))
        psp = ctx.enter_context(tc.tile_pool(name="psp", bufs=1, space="PSUM"))

        # --- constants / weights resident in SBUF ---
        # DMA issue order: first x tile first (so the first projection can
        # start ASAP), then wq/wk, tables, wv, wo.
        wq_s = wpool.tile([128, f, dloc], BF16)
        wk_s = wpool.tile([128, f, dloc], BF16)
        wv_s = wpool.tile([128, f, dloc], BF16)
        wo_s = wpool.tile([128, hpc, Hsz], BF16)
        # startup DMAs ordered by first use: x/wq for the first projection,
        # rc0's rope tables, then wk/wv, the remaining tables, then wo
        xt0 = xpool.tile([128, f, RC], BF16, tag="xt", bufs=2)
        cos_s = cpool.tile([128, Lsz], F32)
        sin_s = cpool.tile([128, Lsz], F32)
        xr = xT.rearrange("(c p) n -> p c n", p=128)
        wqr = wqT.rearrange("(c p) m -> p c m", p=128)
        # consumption-ordered startup: rc0 rope tables first (tiny), then
        # interleaved (x half-A, wq, x half-B) pieces per 4-chunk group so
        # the first projections trickle-start, then wk/wv
        for c4 in range(0, f, 4):
            nc.sync.dma_start(out=xt0[:, c4:c4 + 4, :],
                              in_=xr[:, c4:c4 + 4, 0:RC])
            nc.sync.dma_start(out=wq_s[:, c4:c4 + 4, :],
                              in_=wqr[:, c4:c4 + 4, :])
            if c4 == 0:
                nc.sync.dma_start(out=sin_s[:, 0:RC], in_=sinT[:, 0:RC])
                nc.sync.dma_start(out=cos_s[:, 0:RC], in_=cosT[:, 0:RC])
        wkr = wkT.rearrange("(c p) m -> p c m", p=128)
        for c4 in range(0, f, 4):
            nc.sync.dma_start(out=wk_s[:, c4:c4 + 4, :],
                              in_=wkr[:, c4:c4 + 4, :])
        nc.sync.dma_start(out=wv_s, in_=wvT.rearrange("(c p) m -> p c m", p=128))
        _xt_prefetch = {}
        if Lsz > RC:
            xt1 = xpool.tile([128, f, RC], BF16, tag="xt", bufs=2)
            nc.sync.dma_start(out=xt1, in_=xr[:, :, RC:2 * RC])
            _xt_prefetch[1] = xt1
        if Lsz > RC:
            nc.sync.dma_start(out=sin_s[:, RC:Lsz], in_=sinT[:, RC:Lsz])
            nc.sync.dma_start(out=cos_s[:, RC:Lsz], in_=cosT[:, RC:Lsz])
        nc.sync.dma_start(out=wo_s, in_=woT.rearrange("(h p) n -> p h n", p=128))

        tri_s = cpool.tile([128, 128], BF16)
        make_upper_triangular(nc, tri_s, val=1.0, diag=True)

        # persistent per-sequence activation tensors
        q_s = spool.tile([128, hpc, Lsz], BF16)
        k_s = spool.tile([128, hpc, Lsz], BF16)
        v_s = spool.tile([128, KCL, hpc, HD + 1], BF16)

        for _rep in range(repeat):
            nc.vector.memset(v_s[:, :, :, HD:HD + 1], 1.0)

            # ---------------- P1: qkv projection + rope ----------------
            def rope_apply(p_ps, dst, h, t0, tlen):
                # rope: out_lo = p_lo*cos - p_hi*sin, out_hi = p_hi*cos +
                # p_lo*sin. sin/cos rows repeat at +64, so the half-rotation
                # is done by writing the sin product partition-shifted (legal
                # because the mul reads PSUM: only SBUF+SBUF inputs must
                # share a partition base); the sub/add are then base-aligned.
                ts = slice(t0, t0 + tlen)
                qs_t = work.tile([128, RC], F32, tag="qs", bufs=2)
                nc.vector.tensor_mul(qs_t[0:64, 0:tlen],
                                     p_ps[64:128, 0:tlen],
                                     sin_s[64:128, ts])
                nc.vector.tensor_mul(qs_t[64:128, 0:tlen],
                                     p_ps[0:64, 0:tlen],
                                     sin_s[0:64, ts])
                qc_t = work.tile([128, RC], F32, tag="qc", bufs=2)
                nc.vector.tensor_mul(qc_t[:, 0:tlen], p_ps[:, 0:tlen],
                                     cos_s[:, ts])
                nc.vector.tensor_sub(dst[0:64, h, ts],
                                     qc_t[0:64, 0:tlen], qs_t[0:64, 0:tlen])
                nc.vector.tensor_add(dst[64:128, h, ts],
                                     qc_t[64:128, 0:tlen],
                                     qs_t[64:128, 0:tlen])

            def v_proj(xt, t0):
                # v projection (token-major, all heads at once)
                for m in range(RC // 128):
                    v_ps = psp.tile([128, RC], F32, tag="vy", bufs=3)
                    for c in range(f):
                        nc.tensor.matmul(
                            v_ps[:, 0:dloc],
                            xt[:, c, m * 128:(m + 1) * 128],
                            wv_s[:, c, :],
                            start=(c == 0), stop=(c == f - 1),
                        )
                    kc = t0 // 128 + m
                    nc.scalar.copy(
                        v_s[:, kc, :, 0:HD],
                        v_ps[:, 0:dloc].rearrange("p (h d) -> p h d", h=hpc))

            if phases[0] and _rep == 0:
                # rc0, first rep: chunk-major q/k with 4 concurrent PSUM
                # groups (2 borrowed from the then-idle vy tag) so the PE
                # consumes each interleaved (x piece, w piece) DMA the
                # moment it lands instead of waiting for whole tensors
                for dst, w_s in ((q_s, wq_s), (k_s, wk_s)):
                    pl = [psp.tile([128, RC], F32,
                                   tag=("mm512" if i < 2 else "vy"), bufs=3,
                                   name=f"pp{i}") for i in range(hpc)]
                    for c in range(f):
                        for h in range(hpc):
                            nc.tensor.matmul(
                                pl[h],
                                w_s[:, c, h * 128:(h + 1) * 128],
                                xt0[:, c, :],
                                start=(c == 0), stop=(c == f - 1),
                            )
                    for h in range(hpc):
                        rope_apply(pl[h], dst, h, 0, RC)
                v_proj(xt0, 0)

            rc_start = 1 if _rep == 0 else 0
            for rc in range(rc_start, Lsz // RC if phases[0] else 0):
                t0 = rc * RC
                if _rep == 0 and rc in _xt_prefetch:
                    xt = _xt_prefetch.pop(rc)
                else:
                    xt = xpool.tile([128, f, RC], BF16, tag="xt", bufs=2)
                    nc.sync.dma_start(
                        out=xt,
                        in_=xT.rearrange("(c p) n -> p c n", p=128)[
                            :, :, t0:t0 + RC])

                # q/k projections (head-dim-major) + rope
                for dst, w_s in ((q_s, wq_s), (k_s, wk_s)):
                    for h in range(hpc):
                        p_ps = psp.tile([128, RC], F32, tag="mm512", bufs=3)
                        for c in range(f):
                            nc.tensor.matmul(
                                p_ps,
                                w_s[:, c, h * 128:(h + 1) * 128],
                                xt[:, c, :],
                                start=(c == 0), stop=(c == f - 1),
                            )
                        rope_apply(p_ps, dst, h, t0, RC)
                v_proj(xt, t0)

            # ------- P2+P3: causal attention + interleaved output proj -------
            # oproj work for stripe S is emitted during stripe S+1's attention
            # (PE filler while ACT runs Exp); each emitted group is 4 matmuls
            # into one y PSUM bank + copy + store.
            pending = []

            def drain(n, lag=8):
                # keep ~a stripe's worth queued so oproj never waits on a
                # just-issued transpose
                for _ in range(min(n, len(pending) - lag)):
                    pending.pop(0)()

            def make_group(oT_t, tok, n0):
                def emit():
                    y_ps = psp.tile([128, 512], F32, tag="vy", bufs=3)
                    for h in range(hpc):
                        nc.tensor.matmul(
                            y_ps,
                            oT_t[:, tok % 2, h, :],
                            wo_s[:, h, n0:n0 + 512],
                            start=(h == 0), stop=(h == hpc - 1),
                        )
                    y_t = work.tile([128, 512], F32, tag="yt", bufs=3)
                    # DVE keeps ACT free for Exp (and GPSIMD can't touch PSUM)
                    nc.vector.tensor_copy(y_t, y_ps)
                    nc.sync.dma_start(
                        out=y[tok * 128:(tok + 1) * 128, n0:n0 + 512], in_=y_t)
                return emit

            for qt in range(NST if phases[1] else 0):
                q0 = qt * QT
                npair = qt + 1          # k-chunk pairs in this stripe
                nkc = 2 * npair
                oT_t = spool.tile([128, 2, hpc, 128], BF16, tag="oT", bufs=3)

                # per-h software pipeline: scores+exp for head h overlap the
                # attnV/normalize of head h-1 (so the PE never waits on Exp)
                attnv = []   # deferred closures for the previous head

                def flush_attnv(n):
                    for _ in range(min(n, len(attnv))):
                        attnv.pop(0)()

                def make_attnv(h, es):
                    # attnV+denominator for head h from its exp strip `es`,
                    # then normalize + transpose into oT_t
                    oa = psp.tile([128, HD + 1], F32, tag="oa", bufs=2,
                                  name="oa")
                    steps = []
                    for j in (0, 1):
                        for kc in range(2 * qt + j + 1):
                            def mm(j=j, kc=kc, oa=oa, h=h):
                                nc.tensor.matmul(
                                    oa,
                                    es[:, kc, j * 128:(j + 1) * 128],
                                    v_s[:, kc, h, :],
                                    start=(kc == 0), stop=(kc == 2 * qt + j),
                                )
                            steps.append(mm)

                        def fin(h=h, j=j, oa=oa):
                            rcp = work.tile([128, 1], F32, tag="rcp", bufs=2)
                            nc.vector.reciprocal(rcp, oa[:, HD:HD + 1])
                            o_sb = work.tile([128, 128], BF16, tag="osb",
                                             bufs=2)
                            nc.vector.tensor_scalar_mul(o_sb, oa[:, 0:HD], rcp)
                            nc.sync.dma_start_transpose(
                                out=oT_t[:, j, h, :], in_=o_sb)
                        steps.append(fin)
                        if j == 0:
                            # second subtile needs a fresh accumulator (the
                            # first is still being read by fin)
                            oa = psp.tile([128, HD + 1], F32, tag="oa",
                                          bufs=2, name="oa")
                    return steps

                for h in range(hpc):
                    es = work.tile([128, KCL, QT], BF16, tag="exp", bufs=2)
                    for kp in range(npair):
                        sc = psp.tile([128, 2 * QT], F32, tag="mm512", bufs=3)
                        for i in (0, 1):
                            kc = 2 * kp + i
                            c0 = max(0, kc * 128 - q0)
                            nc.tensor.matmul(
                                sc[:, i * QT + c0:(i + 1) * QT],
                                k_s[:, h, kc * 128:(kc + 1) * 128],
                                q_s[:, h, q0 + c0:q0 + QT],
                                start=True, stop=True,
                            )
                        if kp == npair - 1:
                            # diagonal pair: odd half only live in its last
                            # 128 cols — exp only what the matmuls wrote
                            nc.scalar.activation(es[:, 2 * kp, :], sc[:, 0:QT],
                                                 AF.Exp, scale=scale)
                            nc.scalar.activation(
                                es[:, 2 * kp + 1, 128:QT],
                                sc[:, QT + 128:2 * QT], AF.Exp, scale=scale)
                            nc.gpsimd.tensor_mul(
                                es[:, 2 * kp, 0:128],
                                es[:, 2 * kp, 0:128], tri_s)
                            nc.gpsimd.tensor_mul(
                                es[:, 2 * kp + 1, 128:QT],
                                es[:, 2 * kp + 1, 128:QT], tri_s)
                        else:
                            nc.scalar.activation(
                                es[:, 2 * kp:2 * kp + 2, :],
                                sc.rearrange("p (i n) -> p i n", i=2),
                                AF.Exp, scale=scale)
                        # PE filler while ACT runs Exp: previous head's
                        # attnV chain + one output-projection group
                        flush_attnv(5)
                        drain(1)
                    flush_attnv(len(attnv))
                    attnv = make_attnv(h, es)
                flush_attnv(len(attnv))

                if phases[2]:
                    for tok in (2 * qt, 2 * qt + 1):
                        for n0 in range(0, Hsz, 512):
                            pending.append(make_group(oT_t, tok, n0))
                if qt == NST - 1:
                    drain(len(pending), lag=0)

    nc.compile()
    return nc


# ------------------------- host-side entry point -------------------------

_NC_CACHE = {}


def _get_nc(Lsz, Hsz, hpc, repeat=1):
    key = (Lsz, Hsz, hpc, repeat)
    if key not in _NC_CACHE:
        _NC_CACHE[key] = build_attention_nc(Lsz, Hsz, hpc, repeat=repeat)
    return _NC_CACHE[key]


def make_in_maps(x, w_qkv, w_o):
    """Host-side sharding: per-core input dicts. Core c -> batch c//TP,
    heads [hpc*(c%TP), hpc*(c%TP)+hpc)."""
    import ml_dtypes
    bf16 = ml_dtypes.bfloat16
    Bsz, Lsz, Hsz = x.shape
    hpc = (Hsz // HD) // TP
    dloc = hpc * HD
    xTh = [np.ascontiguousarray(x[b].T).astype(bf16) for b in range(Bsz)]
    w_q, w_k, w_v = (w_qkv[i * Hsz:(i + 1) * Hsz] for i in range(3))
    cosTh, sinTh = rope_tables_T(Lsz)
    in_maps = []
    for c in range(NCORES):
        b, g = divmod(c, TP)
        sl = slice(g * dloc, (g + 1) * dloc)
        in_maps.append({
            "xT": xTh[b],
            "wqT": np.ascontiguousarray(w_q[sl].T).astype(bf16),
            "wkT": np.ascontiguousarray(w_k[sl].T).astype(bf16),
            "wvT": np.ascontiguousarray(w_v[sl].T).astype(bf16),
            "woT": np.ascontiguousarray(w_o[:, sl].T).astype(bf16),
            "cosT": cosTh,
            "sinT": sinTh,
        })
    return in_maps


def run(x, w_qkv, w_o, trace=False):
    x = np.asarray(x)
    Bsz, Lsz, Hsz = x.shape
    hpc = (Hsz // HD) // TP
    nc = _get_nc(Lsz, Hsz, hpc)
    in_maps = make_in_maps(x, np.asarray(w_qkv), np.asarray(w_o))
    res = run_bass_kernel_spmd(nc, in_maps, core_ids=list(range(NCORES)),
                               trace=trace)
    y = np.zeros((Bsz, Lsz, Hsz), dtype=np.float64)
    for c in range(NCORES):
        y[c // TP] += res.results[c]["y"]
    return y.astype(np.float32), res


def kernel(x, w_qkv, w_o):
    y, _ = run(x, w_qkv, w_o, trace=False)
    return y


# revision 37
# speedup vs baseline: 1.0054x; 1.0054x over previous
"""Self-contained Trainium2 Bass kernel: 16-head causal attention with RoPE.

Sharding: DP2 x TP4 — core c handles batch c//4 and heads [4*(c%4), 4*(c%4)+4).
Each core computes the qkv projection for its batch/heads, causal flash
attention, and a partial output projection (w_o columns for its heads); the
4 partial [L, H] outputs per batch are summed on the host.

Key layout/engine strategy:
  - All matmul operands are bf16 (PE streams 1 row/cycle at ANY moving size;
    f32r drops to 4 cycles/row below 256). PSUM accumulation stays fp32, so
    the only precision loss is bf16 operand quantization (~0.4% rel), well
    inside the 2e-2 gate.
  - q, k computed head-dim-major qT/kT [128, L]; v token-major [L, 128] with
    a 129th column of ones.
  - RoPE entirely on DVE: the sin/cos tables repeat rows at +64, so the
    half-rotation is a partition-shifted write of the sin product (legal on
    HW because the multiply reads PSUM; only SBUF+SBUF operand pairs must
    share a partition base), then base-aligned sub/add. No matmul needed.
  - scores computed transposed scT [k_tok, q_tok] = kT_chunk.T @ qT per
    256-token q stripe, two k-chunks packed per PSUM bank so each Exp
    activation covers [128, 512] (amortizes ACT per-instruction overhead).
  - attnV + softmax denominator FUSED: out[q, 0:129] = e_chunk.T @ [v | 1]
    accumulated over k chunks. Column 128 is the denominator, so
    normalization is a per-partition reciprocal + tensor_scalar multiply on
    DVE — no ones-matmul pass, no broadcast matmul.
  - normalized o is transposed back to head-dim-major via an XBAR DMA
    transpose (SBUF->SBUF, no PE or PSUM involvement) so it slots directly
    into the w_o projection as the stationary operand.
  - output projection matmuls are interleaved into the NEXT stripe's
    attention loop: they give the PE work to do while ACT computes Exp.
"""

import numpy as np
from contextlib import ExitStack

import concourse.bass as bass
import concourse.tile as tile
from concourse import bacc, mybir
from concourse.bass_utils import run_bass_kernel_spmd
from concourse.masks import make_upper_triangular

F32 = mybir.dt.float32
BF16 = mybir.dt.bfloat16
AF = mybir.ActivationFunctionType

NCORES = 8
DP = 2          # batch groups
TP = 4          # head groups per batch
HD = 128
ROPE_THETA = 10000.0


def rope_tables_T(Lsz):
    """cos/sin tables transposed to [HD, L], matching the fp32 reference."""
    half = np.arange(0, HD, 2).astype(np.float32) / np.float32(HD)
    inv_freq = (np.float32(1.0) / np.power(np.float32(ROPE_THETA), half,
                                           dtype=np.float32)).astype(np.float32)
    t = np.arange(Lsz, dtype=np.float32)
    freqs = np.outer(t, inv_freq).astype(np.float32)          # [L, HD/2]
    emb = np.concatenate([freqs, freqs], axis=1)              # [L, HD]
    cosT = np.ascontiguousarray(np.cos(emb).astype(np.float32).T)  # [HD, L]
    sinT = np.ascontiguousarray(np.sin(emb).astype(np.float32).T)
    return cosT, sinT


def build_attention_nc(Lsz, Hsz, hpc, repeat=1, phases=(1, 1, 1)):
    """Build + compile the per-core Bass program (identical on all cores).

    Each core: 1 batch of Lsz tokens, hpc heads. repeat>1 re-emits the whole
    computation N times in one program — used only for timing (wall-time
    slope isolates device exec from dispatch overhead)."""
    f = Hsz // 128            # feature chunks of the model dim
    dloc = hpc * HD           # local head dims
    RC = 512                  # token chunk for projection + rope
    QT = 256                  # q stripe for attention (2 x 128 sub-tiles)
    KCL = Lsz // 128          # k chunks per sequence
    NST = Lsz // QT           # stripes
    scale = float(1.0 / np.sqrt(HD))

    nc = bacc.Bacc("TRN2", target_bir_lowering=False, debug=False)

    xT = nc.dram_tensor("xT", [Hsz, Lsz], BF16, kind="ExternalInput").ap()
    wqT = nc.dram_tensor("wqT", [Hsz, dloc], BF16, kind="ExternalInput").ap()
    wkT = nc.dram_tensor("wkT", [Hsz, dloc], BF16, kind="ExternalInput").ap()
    wvT = nc.dram_tensor("wvT", [Hsz, dloc], BF16, kind="ExternalInput").ap()
    woT = nc.dram_tensor("woT", [dloc, Hsz], BF16, kind="ExternalInput").ap()
    cosT = nc.dram_tensor("cosT", [HD, Lsz], F32, kind="ExternalInput").ap()
    sinT = nc.dram_tensor("sinT", [HD, Lsz], F32, kind="ExternalInput").ap()
    y = nc.dram_tensor("y", [Lsz, Hsz], F32, kind="ExternalOutput").ap()

    with tile.TileContext(nc) as tc, \
         nc.allow_low_precision(reason="bf16 matmul operands"), ExitStack() as ctx:
        wpool = ctx.enter_context(tc.tile_pool(name="wpool", bufs=1))
        cpool = ctx.enter_context(tc.tile_pool(name="cpool", bufs=1))
        xpool = ctx.enter_context(tc.tile_pool(name="xpool", bufs=2))
        spool = ctx.enter_context(tc.tile_pool(name="spool", bufs=1))
        work = ctx.enter_context(tc.tile_pool(name="work", bufs=2))
        psp = ctx.enter_context(tc.tile_pool(name="psp", bufs=1, space="PSUM"))

        # --- constants / weights resident in SBUF ---
        wq_s = wpool.tile([128, f, dloc], BF16)
        wk_s = wpool.tile([128, f, dloc], BF16)
        wv_s = wpool.tile([128, f, dloc], BF16)
        wo_s = wpool.tile([128, hpc, Hsz], BF16)
        # startup DMAs ordered by first use: x/wq for the first projection,
        # rc0's rope tables, then wk/wv, the remaining tables, then wo
        xt0 = xpool.tile([128, f, RC], BF16, tag="xt", bufs=2)
        cos_s = cpool.tile([128, Lsz], F32)
        sin_s = cpool.tile([128, Lsz], F32)
        xr = xT.rearrange("(c p) n -> p c n", p=128)
        wqr = wqT.rearrange("(c p) m -> p c m", p=128)
        # consumption-ordered startup: interleaved (x, wq) pieces in
        # graduated sizes so the first chunk-major projection matmuls start
        # ~2us in and consume each piece as it lands; rc0 rope tables ride
        # behind the first piece, then wk pieces, wv, a prefetched rc1 x
        # tile, the remaining tables, and wo last
        for i, (c0, cn) in enumerate(((0, 2), (2, 2), (4, 4), (8, 4),
                                      (12, 4))):
            nc.sync.dma_start(out=xt0[:, c0:c0 + cn, :],
                              in_=xr[:, c0:c0 + cn, 0:RC])
            nc.sync.dma_start(out=wq_s[:, c0:c0 + cn, :],
                              in_=wqr[:, c0:c0 + cn, :])
            if i == 0:
                nc.sync.dma_start(out=sin_s[:, 0:RC], in_=sinT[:, 0:RC])
                nc.sync.dma_start(out=cos_s[:, 0:RC], in_=cosT[:, 0:RC])
        wkr = wkT.rearrange("(c p) m -> p c m", p=128)
        for c4 in range(0, f, 4):
            nc.sync.dma_start(out=wk_s[:, c4:c4 + 4, :],
                              in_=wkr[:, c4:c4 + 4, :])
        nc.sync.dma_start(out=wv_s, in_=wvT.rearrange("(c p) m -> p c m", p=128))
        _xt_prefetch = {}
        if Lsz > RC:
            xt1 = xpool.tile([128, f, RC], BF16, tag="xt", bufs=2)
            nc.sync.dma_start(out=xt1, in_=xr[:, :, RC:2 * RC])
            _xt_prefetch[1] = xt1
        if Lsz > RC:
            nc.sync.dma_start(out=sin_s[:, RC:Lsz], in_=sinT[:, RC:Lsz])
            nc.sync.dma_start(out=cos_s[:, RC:Lsz], in_=cosT[:, RC:Lsz])
        nc.sync.dma_start(out=wo_s, in_=woT.rearrange("(h p) n -> p h n", p=128))

        tri_s = cpool.tile([128, 128], BF16)
        make_upper_triangular(nc, tri_s, val=1.0, diag=True)

        # persistent per-sequence activation tensors
        q_s = spool.tile([128, hpc, Lsz], BF16)
        k_s = spool.tile([128, hpc, Lsz], BF16)
        v_s = spool.tile([128, KCL, hpc, HD + 1], BF16)

        for _rep in range(repeat):
            nc.vector.memset(v_s[:, :, :, HD:HD + 1], 1.0)

            # ---------------- P1: qkv projection + rope ----------------
            def rope_apply(p_ps, dst, h, t0, tlen):
                # rope: out_lo = p_lo*cos - p_hi*sin, out_hi = p_hi*cos +
                # p_lo*sin. sin/cos rows repeat at +64, so the half-rotation
                # is done by writing the sin product partition-shifted (legal
                # because the mul reads PSUM: only SBUF+SBUF inputs must
                # share a partition base); the sub/add are then base-aligned.
                ts = slice(t0, t0 + tlen)
                qs_t = work.tile([128, RC], F32, tag="qs", bufs=2)
                nc.vector.tensor_mul(qs_t[0:64, 0:tlen],
                                     p_ps[64:128, 0:tlen],
                                     sin_s[64:128, ts])
                nc.vector.tensor_mul(qs_t[64:128, 0:tlen],
                                     p_ps[0:64, 0:tlen],
                                     sin_s[0:64, ts])
                qc_t = work.tile([128, RC], F32, tag="qc", bufs=2)
                nc.vector.tensor_mul(qc_t[:, 0:tlen], p_ps[:, 0:tlen],
                                     cos_s[:, ts])
                nc.vector.tensor_sub(dst[0:64, h, ts],
                                     qc_t[0:64, 0:tlen], qs_t[0:64, 0:tlen])
                nc.vector.tensor_add(dst[64:128, h, ts],
                                     qc_t[64:128, 0:tlen],
                                     qs_t[64:128, 0:tlen])

            def v_proj(xt, t0):
                # v projection (token-major, all heads at once)
                for m in range(RC // 128):
                    v_ps = psp.tile([128, RC], F32, tag="vy", bufs=3)
                    for c in range(f):
                        nc.tensor.matmul(
                            v_ps[:, 0:dloc],
                            xt[:, c, m * 128:(m + 1) * 128],
                            wv_s[:, c, :],
                            start=(c == 0), stop=(c == f - 1),
                        )
                    kc = t0 // 128 + m
                    nc.scalar.copy(
                        v_s[:, kc, :, 0:HD],
                        v_ps[:, 0:dloc].rearrange("p (h d) -> p h d", h=hpc))

            if phases[0] and _rep == 0:
                # rc0, first rep: chunk-major q/k with 4 concurrent PSUM
                # groups (2 borrowed from the then-idle vy tag) so the PE
                # consumes each interleaved (x piece, w piece) DMA the
                # moment it lands instead of waiting for whole tensors
                for dst, w_s in ((q_s, wq_s), (k_s, wk_s)):
                    pl = [psp.tile([128, RC], F32,
                                   tag=("mm512" if i < 2 else "vy"), bufs=3,
                                   name=f"pp{i}") for i in range(hpc)]
                    for c in range(f):
                        for h in range(hpc):
                            nc.tensor.matmul(
                                pl[h],
                                w_s[:, c, h * 128:(h + 1) * 128],
                                xt0[:, c, :],
                                start=(c == 0), stop=(c == f - 1),
                            )
                    for h in range(hpc):
                        rope_apply(pl[h], dst, h, 0, RC)
                v_proj(xt0, 0)

            rc_start = 1 if _rep == 0 else 0
            for rc in range(rc_start, Lsz // RC if phases[0] else 0):
                t0 = rc * RC
                if _rep == 0 and rc in _xt_prefetch:
                    xt = _xt_prefetch.pop(rc)
                else:
                    xt = xpool.tile([128, f, RC], BF16, tag="xt", bufs=2)
                    nc.sync.dma_start(
                        out=xt,
                        in_=xT.rearrange("(c p) n -> p c n", p=128)[
                            :, :, t0:t0 + RC])

                # q/k projections (head-dim-major) + rope
                for dst, w_s in ((q_s, wq_s), (k_s, wk_s)):
                    for h in range(hpc):
                        p_ps = psp.tile([128, RC], F32, tag="mm512", bufs=3)
                        for c in range(f):
                            nc.tensor.matmul(
                                p_ps,
                                w_s[:, c, h * 128:(h + 1) * 128],
                                xt[:, c, :],
                                start=(c == 0), stop=(c == f - 1),
                            )
                        rope_apply(p_ps, dst, h, t0, RC)
                v_proj(xt, t0)

            # ------- P2+P3: causal attention + interleaved output proj -------
            # oproj work for stripe S is emitted during stripe S+1's attention
            # (PE filler while ACT runs Exp); each emitted group is 4 matmuls
            # into one y PSUM bank + copy + store.
            pending = []

            def drain(n, lag=8):
                # keep ~a stripe's worth queued so oproj never waits on a
                # just-issued transpose
                for _ in range(min(n, len(pending) - lag)):
                    pending.pop(0)()

            tail_mode = []

            def make_group(oT_t, tok, n0):
                def emit():
                    y_ps = psp.tile([128, 512], F32, tag="vy", bufs=3)
                    for h in range(hpc):
                        nc.tensor.matmul(
                            y_ps,
                            oT_t[:, tok % 2, h, :],
                            wo_s[:, h, n0:n0 + 512],
                            start=(h == 0), stop=(h == hpc - 1),
                        )
                    y_t = work.tile([128, 512], F32, tag="yt", bufs=3)
                    # DVE keeps ACT free for Exp (and GPSIMD can't touch PSUM)
                    nc.vector.tensor_copy(y_t, y_ps)
                    nc.sync.dma_start(
                        out=y[tok * 128:(tok + 1) * 128, n0:n0 + 512], in_=y_t)
                return emit

            for qt in range(NST if phases[1] else 0):
                q0 = qt * QT
                npair = qt + 1          # k-chunk pairs in this stripe
                nkc = 2 * npair
                oT_t = spool.tile([128, 2, hpc, 128], BF16, tag="oT", bufs=3)

                # per-h software pipeline: scores+exp for head h overlap the
                # attnV/normalize of head h-1 (so the PE never waits on Exp)
                attnv = []   # deferred closures for the previous head

                def flush_attnv(n):
                    for _ in range(min(n, len(attnv))):
                        attnv.pop(0)()

                def make_attnv(h, es):
                    # attnV+denominator for head h from its exp strip `es`,
                    # then normalize + transpose into oT_t
                    oa = psp.tile([128, HD + 1], F32, tag="oa", bufs=2,
                                  name="oa")
                    steps = []
                    for j in (0, 1):
                        for kc in range(2 * qt + j + 1):
                            def mm(j=j, kc=kc, oa=oa, h=h):
                                nc.tensor.matmul(
                                    oa,
                                    es[:, kc, j * 128:(j + 1) * 128],
                                    v_s[:, kc, h, :],
                                    start=(kc == 0), stop=(kc == 2 * qt + j),
                                )
                            steps.append(mm)

                        def fin(h=h, j=j, oa=oa):
                            rcp = work.tile([128, 1], F32, tag="rcp", bufs=2)
                            nc.vector.reciprocal(rcp, oa[:, HD:HD + 1])
                            o_sb = work.tile([128, 128], BF16, tag="osb",
                                             bufs=2)
                            nc.vector.tensor_scalar_mul(o_sb, oa[:, 0:HD], rcp)
                            nc.sync.dma_start_transpose(
                                out=oT_t[:, j, h, :], in_=o_sb)
                        steps.append(fin)
                        if j == 0:
                            # second subtile needs a fresh accumulator (the
                            # first is still being read by fin)
                            oa = psp.tile([128, HD + 1], F32, tag="oa",
                                          bufs=2, name="oa")
                    return steps

                for h in range(hpc):
                    es = work.tile([128, KCL, QT], BF16, tag="exp", bufs=2)
                    for kp in range(npair):
                        sc = psp.tile([128, 2 * QT], F32, tag="mm512", bufs=3)
                        for i in (0, 1):
                            kc = 2 * kp + i
                            c0 = max(0, kc * 128 - q0)
                            nc.tensor.matmul(
                                sc[:, i * QT + c0:(i + 1) * QT],
                                k_s[:, h, kc * 128:(kc + 1) * 128],
                                q_s[:, h, q0 + c0:q0 + QT],
                                start=True, stop=True,
                            )
                        if kp == npair - 1:
                            # diagonal pair: odd half only live in its last
                            # 128 cols — exp only what the matmuls wrote
                            nc.scalar.activation(es[:, 2 * kp, :], sc[:, 0:QT],
                                                 AF.Exp, scale=scale)
                            nc.scalar.activation(
                                es[:, 2 * kp + 1, 128:QT],
                                sc[:, QT + 128:2 * QT], AF.Exp, scale=scale)
                            nc.gpsimd.tensor_mul(
                                es[:, 2 * kp, 0:128],
                                es[:, 2 * kp, 0:128], tri_s)
                            nc.gpsimd.tensor_mul(
                                es[:, 2 * kp + 1, 128:QT],
                                es[:, 2 * kp + 1, 128:QT], tri_s)
                        else:
                            nc.scalar.activation(
                                es[:, 2 * kp:2 * kp + 2, :],
                                sc.rearrange("p (i n) -> p i n", i=2),
                                AF.Exp, scale=scale)
                        # PE filler while ACT runs Exp: previous head's
                        # attnV chain + one output-projection group
                        flush_attnv(5)
                        drain(1)
                    flush_attnv(len(attnv))
                    attnv = make_attnv(h, es)
                flush_attnv(len(attnv))

                if phases[2]:
                    for tok in (2 * qt, 2 * qt + 1):
                        for n0 in range(0, Hsz, 512):
                            pending.append(make_group(oT_t, tok, n0))
                if qt == NST - 1:
                    tail_mode.append(1)
                    drain(len(pending), lag=0)

    nc.compile()
    return nc


# ------------------------- host-side entry point -------------------------

_NC_CACHE = {}


def _get_nc(Lsz, Hsz, hpc, repeat=1):
    key = (Lsz, Hsz, hpc, repeat)
    if key not in _NC_CACHE:
        _NC_CACHE[key] = build_attention_nc(Lsz, Hsz, hpc, repeat=repeat)
    return _NC_CACHE[key]


def make_in_maps(x, w_qkv, w_o):
    """Host-side sharding: per-core input dicts. Core c -> batch c//TP,
    heads [hpc*(c%TP), hpc*(c%TP)+hpc)."""
    import ml_dtypes
    bf16 = ml_dtypes.bfloat16
    Bsz, Lsz, Hsz = x.shape
    hpc = (Hsz // HD) // TP
    dloc = hpc * HD
    xTh = [np.ascontiguousarray(x[b].T).astype(bf16) for b in range(Bsz)]
    w_q, w_k, w_v = (w_qkv[i * Hsz:(i + 1) * Hsz] for i in range(3))
    cosTh, sinTh = rope_tables_T(Lsz)
    in_maps = []
    for c in range(NCORES):
        b, g = divmod(c, TP)
        sl = slice(g * dloc, (g + 1) * dloc)
        in_maps.append({
            "xT": xTh[b],
            "wqT": np.ascontiguousarray(w_q[sl].T).astype(bf16),
            "wkT": np.ascontiguousarray(w_k[sl].T).astype(bf16),
            "wvT": np.ascontiguousarray(w_v[sl].T).astype(bf16),
            "woT": np.ascontiguousarray(w_o[:, sl].T).astype(bf16),
            "cosT": cosTh,
            "sinT": sinTh,
        })
    return in_maps


def run(x, w_qkv, w_o, trace=False):
    x = np.asarray(x)
    Bsz, Lsz, Hsz = x.shape
    hpc = (Hsz // HD) // TP
    nc = _get_nc(Lsz, Hsz, hpc)
    in_maps = make_in_maps(x, np.asarray(w_qkv), np.asarray(w_o))
    res = run_bass_kernel_spmd(nc, in_maps, core_ids=list(range(NCORES)),
                               trace=trace)
    y = np.zeros((Bsz, Lsz, Hsz), dtype=np.float64)
    for c in range(NCORES):
        y[c // TP] += res.results[c]["y"]
    return y.astype(np.float32), res


def kernel(x, w_qkv, w_o):
    y, _ = run(x, w_qkv, w_o, trace=False)
    return y


# revision 38
# speedup vs baseline: 1.0219x; 1.0164x over previous
"""Self-contained Trainium2 Bass kernel: 16-head causal attention with RoPE.

Sharding: DP2 x TP4 — core c handles batch c//4 and heads [4*(c%4), 4*(c%4)+4).
Each core computes the qkv projection for its batch/heads, causal flash
attention, and a partial output projection (w_o columns for its heads); the
4 partial [L, H] outputs per batch are summed on the host.

Key layout/engine strategy:
  - All matmul operands are bf16 (PE streams 1 row/cycle at ANY moving size;
    f32r drops to 4 cycles/row below 256). PSUM accumulation stays fp32, so
    the only precision loss is bf16 operand quantization (~0.4% rel), well
    inside the 2e-2 gate.
  - q, k computed head-dim-major qT/kT [128, L]; v token-major [L, 128] with
    a 129th column of ones.
  - RoPE entirely on DVE: the sin/cos tables repeat rows at +64, so the
    half-rotation is a partition-shifted write of the sin product (legal on
    HW because the multiply reads PSUM; only SBUF+SBUF operand pairs must
    share a partition base), then base-aligned sub/add. No matmul needed.
  - scores computed transposed scT [k_tok, q_tok] = kT_chunk.T @ qT per
    256-token q stripe, two k-chunks packed per PSUM bank so each Exp
    activation covers [128, 512] (amortizes ACT per-instruction overhead).
  - attnV + softmax denominator FUSED: out[q, 0:129] = e_chunk.T @ [v | 1]
    accumulated over k chunks. Column 128 is the denominator, so
    normalization is a per-partition reciprocal + tensor_scalar multiply on
    DVE — no ones-matmul pass, no broadcast matmul.
  - normalized o is transposed back to head-dim-major via an XBAR DMA
    transpose (SBUF->SBUF, no PE or PSUM involvement) so it slots directly
    into the w_o projection as the stationary operand.
  - output projection matmuls are interleaved into the NEXT stripe's
    attention loop: they give the PE work to do while ACT computes Exp.
"""

import numpy as np
from contextlib import ExitStack

import concourse.bass as bass
import concourse.tile as tile
from concourse import bacc, mybir
from concourse.bass_utils import run_bass_kernel_spmd
from concourse.masks import make_upper_triangular

F32 = mybir.dt.float32
BF16 = mybir.dt.bfloat16
AF = mybir.ActivationFunctionType

NCORES = 8
DP = 2          # batch groups
TP = 4          # head groups per batch
HD = 128
ROPE_THETA = 10000.0


def rope_tables_T(Lsz):
    """cos/sin tables transposed to [HD, L], matching the fp32 reference."""
    half = np.arange(0, HD, 2).astype(np.float32) / np.float32(HD)
    inv_freq = (np.float32(1.0) / np.power(np.float32(ROPE_THETA), half,
                                           dtype=np.float32)).astype(np.float32)
    t = np.arange(Lsz, dtype=np.float32)
    freqs = np.outer(t, inv_freq).astype(np.float32)          # [L, HD/2]
    emb = np.concatenate([freqs, freqs], axis=1)              # [L, HD]
    cosT = np.ascontiguousarray(np.cos(emb).astype(np.float32).T)  # [HD, L]
    sinT = np.ascontiguousarray(np.sin(emb).astype(np.float32).T)
    return cosT, sinT


def build_attention_nc(Lsz, Hsz, hpc, repeat=1, phases=(1, 1, 1)):
    """Build + compile the per-core Bass program (identical on all cores).

    Each core: 1 batch of Lsz tokens, hpc heads. repeat>1 re-emits the whole
    computation N times in one program — used only for timing (wall-time
    slope isolates device exec from dispatch overhead)."""
    f = Hsz // 128            # feature chunks of the model dim
    dloc = hpc * HD           # local head dims
    RC = 512                  # token chunk for projection + rope
    QT = 256                  # q stripe for attention (2 x 128 sub-tiles)
    KCL = Lsz // 128          # k chunks per sequence
    NST = Lsz // QT           # stripes
    scale = float(1.0 / np.sqrt(HD))

    nc = bacc.Bacc("TRN2", target_bir_lowering=False, debug=False)

    xT = nc.dram_tensor("xT", [Hsz, Lsz], BF16, kind="ExternalInput").ap()
    wqT = nc.dram_tensor("wqT", [Hsz, dloc], BF16, kind="ExternalInput").ap()
    wkT = nc.dram_tensor("wkT", [Hsz, dloc], BF16, kind="ExternalInput").ap()
    wvT = nc.dram_tensor("wvT", [Hsz, dloc], BF16, kind="ExternalInput").ap()
    woT = nc.dram_tensor("woT", [dloc, Hsz], BF16, kind="ExternalInput").ap()
    cosT = nc.dram_tensor("cosT", [HD, Lsz], F32, kind="ExternalInput").ap()
    sinT = nc.dram_tensor("sinT", [HD, Lsz], F32, kind="ExternalInput").ap()
    y = nc.dram_tensor("y", [Lsz, Hsz], F32, kind="ExternalOutput").ap()

    with tile.TileContext(nc) as tc, \
         nc.allow_low_precision(reason="bf16 matmul operands"), ExitStack() as ctx:
        wpool = ctx.enter_context(tc.tile_pool(name="wpool", bufs=1))
        cpool = ctx.enter_context(tc.tile_pool(name="cpool", bufs=1))
        xpool = ctx.enter_context(tc.tile_pool(name="xpool", bufs=2))
        spool = ctx.enter_context(tc.tile_pool(name="spool", bufs=1))
        work = ctx.enter_context(tc.tile_pool(name="work", bufs=2))
        psp = ctx.enter_context(tc.tile_pool(name="psp", bufs=1, space="PSUM"))

        # --- constants / weights resident in SBUF ---
        wq_s = wpool.tile([128, f, dloc], BF16)
        wk_s = wpool.tile([128, f, dloc], BF16)
        wv_s = wpool.tile([128, f, dloc], BF16)
        wo_s = wpool.tile([128, hpc, Hsz], BF16)
        # startup DMAs ordered by first use: x/wq for the first projection,
        # rc0's rope tables, then wk/wv, the remaining tables, then wo
        xt0 = xpool.tile([128, f, RC], BF16, tag="xt", bufs=2)
        cos_s = cpool.tile([128, Lsz], F32)
        sin_s = cpool.tile([128, Lsz], F32)
        xr = xT.rearrange("(c p) n -> p c n", p=128)
        wqr = wqT.rearrange("(c p) m -> p c m", p=128)
        # consumption-ordered startup: interleaved (x, wq) pieces in
        # graduated sizes so the first chunk-major projection matmuls start
        # ~2us in and consume each piece as it lands; rc0 rope tables ride
        # behind the first piece, then wk pieces, wv, a prefetched rc1 x
        # tile, the remaining tables, and wo last
        for i, (c0, cn) in enumerate(((0, 2), (2, 2), (4, 4), (8, 4),
                                      (12, 4))):
            nc.sync.dma_start(out=xt0[:, c0:c0 + cn, :],
                              in_=xr[:, c0:c0 + cn, 0:RC])
            nc.sync.dma_start(out=wq_s[:, c0:c0 + cn, :],
                              in_=wqr[:, c0:c0 + cn, :])
            if i == 0:
                nc.sync.dma_start(out=sin_s[:, 0:RC], in_=sinT[:, 0:RC])
                nc.sync.dma_start(out=cos_s[:, 0:RC], in_=cosT[:, 0:RC])
        wkr = wkT.rearrange("(c p) m -> p c m", p=128)
        for c4 in range(0, f, 4):
            nc.sync.dma_start(out=wk_s[:, c4:c4 + 4, :],
                              in_=wkr[:, c4:c4 + 4, :])
        nc.sync.dma_start(out=wv_s, in_=wvT.rearrange("(c p) m -> p c m", p=128))
        _xt_prefetch = {}
        if Lsz > RC:
            xt1 = xpool.tile([128, f, RC], BF16, tag="xt", bufs=2)
            nc.sync.dma_start(out=xt1, in_=xr[:, :, RC:2 * RC])
            _xt_prefetch[1] = xt1
        if Lsz > RC:
            nc.sync.dma_start(out=sin_s[:, RC:Lsz], in_=sinT[:, RC:Lsz])
            nc.sync.dma_start(out=cos_s[:, RC:Lsz], in_=cosT[:, RC:Lsz])
        nc.sync.dma_start(out=wo_s, in_=woT.rearrange("(h p) n -> p h n", p=128))

        tri_s = cpool.tile([128, 128], BF16)
        make_upper_triangular(nc, tri_s, val=1.0, diag=True)

        # persistent per-sequence activation tensors
        q_s = spool.tile([128, hpc, Lsz], BF16)
        k_s = spool.tile([128, hpc, Lsz], BF16)
        v_s = spool.tile([128, KCL, hpc, HD + 1], BF16)

        for _rep in range(repeat):
            nc.vector.memset(v_s[:, :, :, HD:HD + 1], 1.0)

            # ---------------- P1: qkv projection + rope ----------------
            def rope_apply(p_ps, dst, h, t0, tlen):
                # rope: out_lo = p_lo*cos - p_hi*sin, out_hi = p_hi*cos +
                # p_lo*sin. sin/cos rows repeat at +64, so the half-rotation
                # is done by writing the sin product partition-shifted (legal
                # because the mul reads PSUM: only SBUF+SBUF inputs must
                # share a partition base); the sub/add are then base-aligned.
                ts = slice(t0, t0 + tlen)
                qs_t = work.tile([128, RC], F32, tag="qs", bufs=2)
                nc.vector.tensor_mul(qs_t[0:64, 0:tlen],
                                     p_ps[64:128, 0:tlen],
                                     sin_s[64:128, ts])
                nc.vector.tensor_mul(qs_t[64:128, 0:tlen],
                                     p_ps[0:64, 0:tlen],
                                     sin_s[0:64, ts])
                qc_t = work.tile([128, RC], F32, tag="qc", bufs=2)
                nc.vector.tensor_mul(qc_t[:, 0:tlen], p_ps[:, 0:tlen],
                                     cos_s[:, ts])
                nc.vector.tensor_sub(dst[0:64, h, ts],
                                     qc_t[0:64, 0:tlen], qs_t[0:64, 0:tlen])
                nc.vector.tensor_add(dst[64:128, h, ts],
                                     qc_t[64:128, 0:tlen],
                                     qs_t[64:128, 0:tlen])

            vfill = []   # deferred PE work used as early-attention filler

            def v_proj(xt, t0, defer=False):
                # v projection (token-major, all heads at once). The last
                # chunk's v is first read at stripe ~6, so its matmuls can be
                # deferred into the early attention stripes, where the PE
                # otherwise waits on the first Exp strips.
                def one(m, xt=xt, t0=t0):
                    v_ps = psp.tile([128, RC], F32, tag="vy", bufs=3)
                    for c in range(f):
                        nc.tensor.matmul(
                            v_ps[:, 0:dloc],
                            xt[:, c, m * 128:(m + 1) * 128],
                            wv_s[:, c, :],
                            start=(c == 0), stop=(c == f - 1),
                        )
                    kc = t0 // 128 + m
                    nc.scalar.copy(
                        v_s[:, kc, :, 0:HD],
                        v_ps[:, 0:dloc].rearrange("p (h d) -> p h d", h=hpc))
                for m in range(RC // 128):
                    if defer:
                        vfill.append(lambda m=m: one(m))
                    else:
                        one(m)

            if phases[0] and _rep == 0:
                # rc0, first rep: chunk-major q/k with 4 concurrent PSUM
                # groups (2 borrowed from the then-idle vy tag) so the PE
                # consumes each interleaved (x piece, w piece) DMA the
                # moment it lands instead of waiting for whole tensors
                for dst, w_s in ((q_s, wq_s), (k_s, wk_s)):
                    pl = [psp.tile([128, RC], F32,
                                   tag=("mm512" if i < 2 else "vy"), bufs=3,
                                   name=f"pp{i}") for i in range(hpc)]
                    for c in range(f):
                        for h in range(hpc):
                            nc.tensor.matmul(
                                pl[h],
                                w_s[:, c, h * 128:(h + 1) * 128],
                                xt0[:, c, :],
                                start=(c == 0), stop=(c == f - 1),
                            )
                    for h in range(hpc):
                        rope_apply(pl[h], dst, h, 0, RC)
                v_proj(xt0, 0)

            rc_start = 1 if _rep == 0 else 0
            for rc in range(rc_start, Lsz // RC if phases[0] else 0):
                t0 = rc * RC
                if _rep == 0 and rc in _xt_prefetch:
                    xt = _xt_prefetch.pop(rc)
                else:
                    xt = xpool.tile([128, f, RC], BF16, tag="xt", bufs=2)
                    nc.sync.dma_start(
                        out=xt,
                        in_=xT.rearrange("(c p) n -> p c n", p=128)[
                            :, :, t0:t0 + RC])

                # q/k projections (head-dim-major) + rope
                for dst, w_s in ((q_s, wq_s), (k_s, wk_s)):
                    for h in range(hpc):
                        p_ps = psp.tile([128, RC], F32, tag="mm512", bufs=3)
                        for c in range(f):
                            nc.tensor.matmul(
                                p_ps,
                                w_s[:, c, h * 128:(h + 1) * 128],
                                xt[:, c, :],
                                start=(c == 0), stop=(c == f - 1),
                            )
                        rope_apply(p_ps, dst, h, t0, RC)
                v_proj(xt, t0, defer=(rc == Lsz // RC - 1 and rc > 0))

            # ------- P2+P3: causal attention + interleaved output proj -------
            # oproj work for stripe S is emitted during stripe S+1's attention
            # (PE filler while ACT runs Exp); each emitted group is 4 matmuls
            # into one y PSUM bank + copy + store.
            pending = []

            def drain(n, lag=8):
                # deferred v-projection first (deps long since ready), then
                # oproj with ~a stripe's worth queued so it never waits on a
                # just-issued transpose
                for _ in range(n):
                    if vfill:
                        vfill.pop(0)()
                    elif len(pending) > lag:
                        pending.pop(0)()
                    else:
                        break

            tail_mode = []

            def make_group(oT_t, tok, n0):
                def emit():
                    y_ps = psp.tile([128, 512], F32, tag="vy", bufs=3)
                    for h in range(hpc):
                        nc.tensor.matmul(
                            y_ps,
                            oT_t[:, tok % 2, h, :],
                            wo_s[:, h, n0:n0 + 512],
                            start=(h == 0), stop=(h == hpc - 1),
                        )
                    y_t = work.tile([128, 512], F32, tag="yt", bufs=3)
                    # DVE keeps ACT free for Exp (and GPSIMD can't touch PSUM)
                    nc.vector.tensor_copy(y_t, y_ps)
                    nc.sync.dma_start(
                        out=y[tok * 128:(tok + 1) * 128, n0:n0 + 512], in_=y_t)
                return emit

            for qt in range(NST if phases[1] else 0):
                q0 = qt * QT
                npair = qt + 1          # k-chunk pairs in this stripe
                nkc = 2 * npair
                oT_t = spool.tile([128, 2, hpc, 128], BF16, tag="oT", bufs=3)

                # per-h software pipeline: scores+exp for head h overlap the
                # attnV/normalize of head h-1 (so the PE never waits on Exp)
                attnv = []   # deferred closures for the previous head

                def flush_attnv(n):
                    for _ in range(min(n, len(attnv))):
                        attnv.pop(0)()

                def make_attnv(h, es):
                    # attnV+denominator for head h from its exp strip `es`,
                    # then normalize + transpose into oT_t
                    oa = psp.tile([128, HD + 1], F32, tag="oa", bufs=2,
                                  name="oa")
                    steps = []
                    for j in (0, 1):
                        for kc in range(2 * qt + j + 1):
                            def mm(j=j, kc=kc, oa=oa, h=h):
                                nc.tensor.matmul(
                                    oa,
                                    es[:, kc, j * 128:(j + 1) * 128],
                                    v_s[:, kc, h, :],
                                    start=(kc == 0), stop=(kc == 2 * qt + j),
                                )
                            steps.append(mm)

                        def fin(h=h, j=j, oa=oa):
                            rcp = work.tile([128, 1], F32, tag="rcp", bufs=2)
                            nc.vector.reciprocal(rcp, oa[:, HD:HD + 1])
                            o_sb = work.tile([128, 128], BF16, tag="osb",
                                             bufs=2)
                            nc.vector.tensor_scalar_mul(o_sb, oa[:, 0:HD], rcp)
                            nc.sync.dma_start_transpose(
                                out=oT_t[:, j, h, :], in_=o_sb)
                        steps.append(fin)
                        if j == 0:
                            # second subtile needs a fresh accumulator (the
                            # first is still being read by fin)
                            oa = psp.tile([128, HD + 1], F32, tag="oa",
                                          bufs=2, name="oa")
                    return steps

                for h in range(hpc):
                    es = work.tile([128, KCL, QT], BF16, tag="exp", bufs=2)
                    for kp in range(npair):
                        sc = psp.tile([128, 2 * QT], F32, tag="mm512", bufs=3)
                        for i in (0, 1):
                            kc = 2 * kp + i
                            c0 = max(0, kc * 128 - q0)
                            nc.tensor.matmul(
                                sc[:, i * QT + c0:(i + 1) * QT],
                                k_s[:, h, kc * 128:(kc + 1) * 128],
                                q_s[:, h, q0 + c0:q0 + QT],
                                start=True, stop=True,
                            )
                        if kp == npair - 1:
                            # diagonal pair: odd half only live in its last
                            # 128 cols — exp only what the matmuls wrote
                            nc.scalar.activation(es[:, 2 * kp, :], sc[:, 0:QT],
                                                 AF.Exp, scale=scale)
                            nc.scalar.activation(
                                es[:, 2 * kp + 1, 128:QT],
                                sc[:, QT + 128:2 * QT], AF.Exp, scale=scale)
                            nc.gpsimd.tensor_mul(
                                es[:, 2 * kp, 0:128],
                                es[:, 2 * kp, 0:128], tri_s)
                            nc.gpsimd.tensor_mul(
                                es[:, 2 * kp + 1, 128:QT],
                                es[:, 2 * kp + 1, 128:QT], tri_s)
                        else:
                            nc.scalar.activation(
                                es[:, 2 * kp:2 * kp + 2, :],
                                sc.rearrange("p (i n) -> p i n", i=2),
                                AF.Exp, scale=scale)
                        # PE filler while ACT runs Exp: previous head's
                        # attnV chain + one output-projection group
                        flush_attnv(5)
                        drain(1)
                    flush_attnv(len(attnv))
                    attnv = make_attnv(h, es)
                flush_attnv(len(attnv))

                if phases[2]:
                    for tok in (2 * qt, 2 * qt + 1):
                        for n0 in range(0, Hsz, 512):
                            pending.append(make_group(oT_t, tok, n0))
                if qt == NST - 1:
                    tail_mode.append(1)
                    drain(len(pending) + len(vfill), lag=0)

    nc.compile()
    return nc


# ------------------------- host-side entry point -------------------------

_NC_CACHE = {}


def _get_nc(Lsz, Hsz, hpc, repeat=1):
    key = (Lsz, Hsz, hpc, repeat)
    if key not in _NC_CACHE:
        _NC_CACHE[key] = build_attention_nc(Lsz, Hsz, hpc, repeat=repeat)
    return _NC_CACHE[key]


def make_in_maps(x, w_qkv, w_o):
    """Host-side sharding: per-core input dicts. Core c -> batch c//TP,
    heads [hpc*(c%TP), hpc*(c%TP)+hpc)."""
    import ml_dtypes
    bf16 = ml_dtypes.bfloat16
    Bsz, Lsz, Hsz = x.shape
    hpc = (Hsz // HD) // TP
    dloc = hpc * HD
    xTh = [np.ascontiguousarray(x[b].T).astype(bf16) for b in range(Bsz)]
    w_q, w_k, w_v = (w_qkv[i * Hsz:(i + 1) * Hsz] for i in range(3))
    cosTh, sinTh = rope_tables_T(Lsz)
    in_maps = []
    for c in range(NCORES):
        b, g = divmod(c, TP)
        sl = slice(g * dloc, (g + 1) * dloc)
        in_maps.append({
            "xT": xTh[b],
            "wqT": np.ascontiguousarray(w_q[sl].T).astype(bf16),
            "wkT": np.ascontiguousarray(w_k[sl].T).astype(bf16),
            "wvT": np.ascontiguousarray(w_v[sl].T).astype(bf16),
            "woT": np.ascontiguousarray(w_o[:, sl].T).astype(bf16),
            "cosT": cosTh,
            "sinT": sinTh,
        })
    return in_maps


def run(x, w_qkv, w_o, trace=False):
    x = np.asarray(x)
    Bsz, Lsz, Hsz = x.shape
    hpc = (Hsz // HD) // TP
    nc = _get_nc(Lsz, Hsz, hpc)
    in_maps = make_in_maps(x, np.asarray(w_qkv), np.asarray(w_o))
    res = run_bass_kernel_spmd(nc, in_maps, core_ids=list(range(NCORES)),
                               trace=trace)
    y = np.zeros((Bsz, Lsz, Hsz), dtype=np.float64)
    for c in range(NCORES):
        y[c // TP] += res.results[c]["y"]
    return y.astype(np.float32), res


def kernel(x, w_qkv, w_o):
    y, _ = run(x, w_qkv, w_o, trace=False)
    return y
